# revision 1
# baseline (speedup 1.0000x reference)
"""NeuralODE Trainium2 kernel.

Math being implemented (see the reference nn.Module):
    h0  = (x[:, -1, :] @ Wi) + bi                  # only the LAST timestep of x
                                                   # ever affects the output
    dh/dt = tanh(h @ W1 + b1) @ W2 + b2            # autonomous MLP vector field
    h(T)  = integrate over T = S-1 with a Runge-Kutta scheme
    out = h(T) @ Wo + bo                           # [B, 1]

The reference integrates with 64 fixed dopri5 steps.  The field is very mild
(||W2||_2 ~ 0.02, so the step Lipschitz number dt*L << 1): a dopri5 integrator
with NSTEPS=8 steps reproduces the reference output to ~3e-7 relative error
(the fp32 noise floor of the reference itself); NSTEPS=4 gives ~5e-6.

Kernel layout (pure data parallel over batch, 8 cores, 512 rows each):
  * All tensors live TRANSPOSED on chip: state y^T is [H=128 partitions, B=512].
    Then every matmul "z @ M" becomes nc.tensor.matmul(out, lhsT=M, rhs=z^T)
    and biases become per-partition scalars (free on ACT/DVE).
  * All RK stage combinations are folded into PSUM-accumulated matmuls:
      v_i = z_i @ W1  with z_i = y + dt*sum_j a_ij k_j  is built by telescoping
      v_i = v_{i-1} + sum_j dt*(a_ij - a_{i-1,j}) * (t_j @ (W2 @ W1))  in ONE
      accumulating PSUM bank, where t_j = tanh(v_j + beta_j).  The constant
      b2-corrections fold into the per-stage tanh biases beta_i = b1 + dt*c_i*(W1^T b2).
    So the vector engine does almost nothing; PE + ACT carry the step.
  * y update: y += (sum_j dt*b_j t_j) @ W2 + dt*b2, with the b_j folded into
    pre-scaled W2 copies accumulated in a second PSUM bank.
All scaled weight copies / bias vectors are computed ON DEVICE in a preamble
from the raw weights (host only slices, transposes and reshapes inputs).
"""

import os
from contextlib import ExitStack

import numpy as np

import concourse.bass as bass
import concourse.tile as tile
from concourse import bacc, mybir
from concourse.bass_utils import run_bass_kernel_spmd

F32 = mybir.dt.float32
NCORES = 8
B, S, D, H = 4096, 200, 64, 128
BS = B // NCORES  # 512 batch rows per core

NSTEPS = int(os.environ.get("ODE_NSTEPS", "8"))

# dopri5 tableau
_A = [
    (),
    (1 / 5,),
    (3 / 40, 9 / 40),
    (44 / 45, -56 / 15, 32 / 9),
    (19372 / 6561, -25360 / 2187, 64448 / 6561, -212 / 729),
    (9017 / 3168, -355 / 33, 46732 / 5247, 49 / 176, -5103 / 18656),
]
_BW = (35 / 384, 0.0, 500 / 1113, 125 / 192, -2187 / 6784, 11 / 84)
_C = [sum(a) for a in _A]  # consistency: c_i = sum_j a_ij


def _build(nsteps: int) -> bass.Bass:
    nc = bacc.Bacc()
    dt = float(S - 1) / nsteps
    Tanh = mybir.ActivationFunctionType.Tanh

    xT = nc.declare_dram_parameter("xT", [D, BS], F32, isOutput=False)
    wi = nc.declare_dram_parameter("Wi", [D, H], F32, isOutput=False)
    w1 = nc.declare_dram_parameter("W1", [H, H], F32, isOutput=False)
    w2 = nc.declare_dram_parameter("W2", [H, H], F32, isOutput=False)
    w2t = nc.declare_dram_parameter("W2T", [H, H], F32, isOutput=False)
    bi = nc.declare_dram_parameter("bi", [H, 1], F32, isOutput=False)
    b1 = nc.declare_dram_parameter("b1", [H, 1], F32, isOutput=False)
    b2 = nc.declare_dram_parameter("b2", [H, 1], F32, isOutput=False)
    wo = nc.declare_dram_parameter("Wo", [H, 1], F32, isOutput=False)
    bo = nc.declare_dram_parameter("bo", [1, 1], F32, isOutput=False)
    out = nc.declare_dram_parameter("out", [1, BS], F32, isOutput=True)

    with tile.TileContext(nc) as tc, ExitStack() as ctx:
        const = ctx.enter_context(tc.tile_pool(name="const", bufs=1))
        state = ctx.enter_context(tc.tile_pool(name="state", bufs=2))
        tpool = ctx.enter_context(tc.tile_pool(name="tpool", bufs=2))
        psum = ctx.enter_context(tc.tile_pool(name="psum", bufs=2, space="PSUM"))
        psmall = ctx.enter_context(tc.tile_pool(name="psmall", bufs=1, space="PSUM"))

        def load(name, dram, shape):
            t = const.tile(shape, F32, tag=name)
            nc.sync.dma_start(t[:], dram[:])
            return t

        sWi = load("sWi", wi, [D, H])
        sW1 = load("sW1", w1, [H, H])
        sW2 = load("sW2", w2, [H, H])
        sW2T = load("sW2T", w2t, [H, H])
        sxT = load("sxT", xT, [D, BS])
        sbi = load("sbi", bi, [H, 1])
        sb1 = load("sb1", b1, [H, 1])
        sb2 = load("sb2", b2, [H, 1])
        sWo = load("sWo", wo, [H, 1])
        sbo = load("sbo", bo, [1, 1])

        # W21 = W2 @ W1 in lhsT orientation [H_in(t-space), H_out]
        pW21 = psmall.tile([H, H], F32, tag="pW21")
        nc.tensor.matmul(pW21[:], sW2T[:], sW1[:], start=True, stop=True)

        # Telescoped stage-combination weights G[i][j] = dt*(a_ij - a_{i-1,j})*W21
        G = {}
        for i in range(2, 7):
            row, prev = _A[i - 1], _A[i - 2]
            for j in range(1, i):
                d = row[j - 1] - (prev[j - 1] if j - 1 < len(prev) else 0.0)
                g = const.tile([H, H], F32, tag=f"G{i}{j}")
                nc.vector.tensor_scalar_mul(g[:], pW21[:], float(dt * d))
                G[(i, j)] = g

        # Final-combination weights dt*b_j*W2 (b_2 == 0 -> skipped)
        W2b = {}
        for j in (1, 3, 4, 5, 6):
            t = const.tile([H, H], F32, tag=f"W2b{j}")
            nc.vector.tensor_scalar_mul(t[:], sW2[:], float(dt * _BW[j - 1]))
            W2b[j] = t

        # cb = W1^T b2; per-stage tanh biases beta_i = b1 + dt*c_i*cb
        pcb = psmall.tile([H, 1], F32, tag="pcb")
        nc.tensor.matmul(pcb[:], sW1[:], sb2[:], start=True, stop=True)
        betas = [sb1]
        for i in range(2, 7):
            tmp = const.tile([H, 1], F32, tag=f"cbt{i}")
            nc.vector.tensor_scalar_mul(tmp[:], pcb[:], float(dt * _C[i - 1]))
            bt = const.tile([H, 1], F32, tag=f"beta{i}")
            nc.vector.tensor_add(bt[:], tmp[:], sb1[:])
            betas.append(bt)
        sbf = const.tile([H, 1], F32, tag="sbf")  # dt*b2 for the y update
        nc.vector.tensor_scalar_mul(sbf[:], sb2[:], float(dt))

        # h0 = x_last @ Wi + bi  (transposed: [H, BS])
        ph = psum.tile([H, BS], F32, tag="V")
        nc.tensor.matmul(ph[:], sWi[:], sxT[:], start=True, stop=True)
        y = state.tile([H, BS], F32, tag="y")
        nc.vector.tensor_scalar_add(y[:], ph[:], sbi[:])

        for _ in range(nsteps):
            V = psum.tile([H, BS], F32, tag="V")
            nc.tensor.matmul(V[:], sW1[:], y[:], start=True, stop=False)
            ts = []
            for i in range(1, 7):
                ti = tpool.tile([H, BS], F32, tag=f"t{i}")
                nc.scalar.activation(ti[:], V[:], Tanh, bias=betas[i - 1][:])
                ts.append(ti)
                if i < 6:
                    for j in range(1, i + 1):
                        last = i + 1 == 6 and j == i
                        nc.tensor.matmul(
                            V[:], G[(i + 1, j)][:], ts[j - 1][:],
                            start=False, stop=last,
                        )
            P2 = psum.tile([H, BS], F32, tag="P2")
            for n, j in enumerate((1, 3, 4, 5, 6)):
                nc.tensor.matmul(
                    P2[:], W2b[j][:], ts[j - 1][:], start=(n == 0), stop=(j == 6)
                )
            tmp = tpool.tile([H, BS], F32, tag="ktmp")
            nc.vector.tensor_scalar_add(tmp[:], P2[:], sbf[:])
            y2 = state.tile([H, BS], F32, tag="y")
            nc.vector.tensor_add(y2[:], y[:], tmp[:])
            y = y2

        # out = y @ Wo + bo  (transposed: [1, BS])
        po = psmall.tile([1, BS], F32, tag="po")
        nc.tensor.matmul(po[:], sWo[:], y[:], start=True, stop=True)
        so = const.tile([1, BS], F32, tag="so")
        nc.vector.tensor_scalar_add(so[:], po[:], sbo[:])
        nc.sync.dma_start(out[:], so[:])

    nc.finalize()
    return nc


_NC_CACHE: dict[int, bass.Bass] = {}


def _get_nc(nsteps: int = NSTEPS) -> bass.Bass:
    if nsteps not in _NC_CACHE:
        _NC_CACHE[nsteps] = _build(nsteps)
    return _NC_CACHE[nsteps]


def _in_maps(x, Wi, bi, W1, b1, W2, b2, Wo, bo):
    f = lambda a: np.ascontiguousarray(np.asarray(a), dtype=np.float32)
    x_lastT = f(np.asarray(x)[:, -1, :].T)  # [D, B]
    Wi, W1, W2, Wo = f(Wi), f(W1), f(W2), f(Wo)
    W2T = f(W2.T)
    bi_c, b1_c, b2_c = (f(v).reshape(H, 1) for v in (bi, b1, b2))
    Wo_c = Wo.reshape(H, 1)
    bo_c = f(bo).reshape(1, 1)
    maps = []
    for c in range(NCORES):
        maps.append({
            "xT": f(x_lastT[:, c * BS:(c + 1) * BS]),
            "Wi": Wi, "W1": W1, "W2": W2, "W2T": W2T,
            "bi": bi_c, "b1": b1_c, "b2": b2_c, "Wo": Wo_c, "bo": bo_c,
        })
    return maps


def kernel(x, Wi, bi, W1, b1, W2, b2, Wo, bo, _trace=False, _trace_kw=None):
    nc = _get_nc()
    maps = _in_maps(x, Wi, bi, W1, b1, W2, b2, Wo, bo)
    res = run_bass_kernel_spmd(
        nc, maps, list(range(NCORES)), trace=_trace, **(_trace_kw or {})
    )
    outv = np.concatenate([res.results[c]["out"][0] for c in range(NCORES)])
    out = outv.reshape(B, 1).astype(np.float32)
    if _trace:
        return out, res
    return out


# revision 2
# speedup vs baseline: 1.8367x; 1.8367x over previous
"""NeuralODE Trainium2 kernel.

Math being implemented (see the reference nn.Module):
    h0  = (x[:, -1, :] @ Wi) + bi                  # only the LAST timestep of x
                                                   # ever affects the output
    dh/dt = tanh(h @ W1 + b1) @ W2 + b2            # autonomous MLP vector field
    h(T)  = integrate over T = S-1 with a Runge-Kutta scheme
    out = h(T) @ Wo + bo                           # [B, 1]

The reference integrates with 64 fixed dopri5 steps.  The field is very mild
(||W2||_2 ~ 0.02, so the step Lipschitz number dt*L << 1): a dopri5 integrator
with NSTEPS=8 steps reproduces the reference output to ~3e-7 relative error
(the fp32 noise floor of the reference itself); NSTEPS=4 gives ~5e-6.

Kernel layout (pure data parallel over batch, 8 cores, 512 rows each):
  * All tensors live TRANSPOSED on chip: state y^T is [H=128 partitions, B=512].
    Then every matmul "z @ M" becomes nc.tensor.matmul(out, lhsT=M, rhs=z^T)
    and biases become per-partition scalars (free on ACT/DVE).
  * All RK stage combinations are folded into PSUM-accumulated matmuls:
      v_i = z_i @ W1  with z_i = y + dt*sum_j a_ij k_j  is built by telescoping
      v_i = v_{i-1} + sum_j dt*(a_ij - a_{i-1,j}) * (t_j @ (W2 @ W1))  in ONE
      accumulating PSUM bank, where t_j = tanh(v_j + beta_j).  The constant
      b2-corrections fold into the per-stage tanh biases beta_i = b1 + dt*c_i*(W1^T b2).
    So the vector engine does almost nothing; PE + ACT carry the step.
  * y update: y += (sum_j dt*b_j t_j) @ W2 + dt*b2, with the b_j folded into
    pre-scaled W2 copies accumulated in a second PSUM bank.
All scaled weight copies / bias vectors are computed ON DEVICE in a preamble
from the raw weights (host only slices, transposes and reshapes inputs).
"""

import os
from contextlib import ExitStack

import numpy as np

import concourse.bass as bass
import concourse.tile as tile
from concourse import bacc, mybir
from concourse.bass_utils import run_bass_kernel_spmd

F32 = mybir.dt.float32
NCORES = 8
B, S, D, H = 4096, 200, 64, 128
BS = B // NCORES  # 512 batch rows per core

NSTEPS = int(os.environ.get("ODE_NSTEPS", "8"))
MM_DTYPE = os.environ.get("MM_DTYPE", "f32r")  # f32r: 1 cyc/row PE vs 4 for f32

# dopri5 tableau
_A = [
    (),
    (1 / 5,),
    (3 / 40, 9 / 40),
    (44 / 45, -56 / 15, 32 / 9),
    (19372 / 6561, -25360 / 2187, 64448 / 6561, -212 / 729),
    (9017 / 3168, -355 / 33, 46732 / 5247, 49 / 176, -5103 / 18656),
]
_BW = (35 / 384, 0.0, 500 / 1113, 125 / 192, -2187 / 6784, 11 / 84)
_C = [sum(a) for a in _A]  # consistency: c_i = sum_j a_ij


def _build(nsteps: int) -> bass.Bass:
    nc = bacc.Bacc()
    mmdt = mybir.dt.float32r if MM_DTYPE == "f32r" else None
    R = (lambda ap: ap.bitcast(mmdt)) if mmdt else (lambda ap: ap)
    dt = float(S - 1) / nsteps
    Tanh = mybir.ActivationFunctionType.Tanh

    xT = nc.declare_dram_parameter("xT", [D, BS], F32, isOutput=False)
    wi = nc.declare_dram_parameter("Wi", [D, H], F32, isOutput=False)
    w1 = nc.declare_dram_parameter("W1", [H, H], F32, isOutput=False)
    w2 = nc.declare_dram_parameter("W2", [H, H], F32, isOutput=False)
    w2t = nc.declare_dram_parameter("W2T", [H, H], F32, isOutput=False)
    bi = nc.declare_dram_parameter("bi", [H, 1], F32, isOutput=False)
    b1 = nc.declare_dram_parameter("b1", [H, 1], F32, isOutput=False)
    b2 = nc.declare_dram_parameter("b2", [H, 1], F32, isOutput=False)
    wo = nc.declare_dram_parameter("Wo", [H, 1], F32, isOutput=False)
    bo = nc.declare_dram_parameter("bo", [1, 1], F32, isOutput=False)
    out = nc.declare_dram_parameter("out", [1, BS], F32, isOutput=True)

    with tile.TileContext(nc) as tc, ExitStack() as ctx:
        const = ctx.enter_context(tc.tile_pool(name="const", bufs=1))
        state = ctx.enter_context(tc.tile_pool(name="state", bufs=2))
        tpool = ctx.enter_context(tc.tile_pool(name="tpool", bufs=2))
        psum = ctx.enter_context(tc.tile_pool(name="psum", bufs=2, space="PSUM"))
        psmall = ctx.enter_context(tc.tile_pool(name="psmall", bufs=1, space="PSUM"))

        def load(name, dram, shape):
            t = const.tile(shape, F32, tag=name)
            nc.sync.dma_start(t[:], dram[:])
            return t

        sWi = load("sWi", wi, [D, H])
        sW1 = load("sW1", w1, [H, H])
        sW2 = load("sW2", w2, [H, H])
        sW2T = load("sW2T", w2t, [H, H])
        sxT = load("sxT", xT, [D, BS])
        sbi = load("sbi", bi, [H, 1])
        sb1 = load("sb1", b1, [H, 1])
        sb2 = load("sb2", b2, [H, 1])
        sWo = load("sWo", wo, [H, 1])
        sbo = load("sbo", bo, [1, 1])

        # W21 = W2 @ W1 in lhsT orientation [H_in(t-space), H_out]
        pW21 = psmall.tile([H, H], F32, tag="pW21")
        nc.tensor.matmul(pW21[:], R(sW2T[:]), R(sW1[:]), start=True, stop=True)

        # Telescoped stage-combination weights G[i][j] = dt*(a_ij - a_{i-1,j})*W21
        G = {}
        for i in range(2, 7):
            row, prev = _A[i - 1], _A[i - 2]
            for j in range(1, i):
                d = row[j - 1] - (prev[j - 1] if j - 1 < len(prev) else 0.0)
                g = const.tile([H, H], F32, tag=f"G{i}{j}")
                nc.vector.tensor_scalar_mul(g[:], pW21[:], float(dt * d))
                G[(i, j)] = g

        # Final-combination weights dt*b_j*W2 (b_2 == 0 -> skipped)
        W2b = {}
        for j in (1, 3, 4, 5, 6):
            t = const.tile([H, H], F32, tag=f"W2b{j}")
            nc.vector.tensor_scalar_mul(t[:], sW2[:], float(dt * _BW[j - 1]))
            W2b[j] = t

        # cb = W1^T b2; per-stage tanh biases beta_i = b1 + dt*c_i*cb
        pcb = psmall.tile([H, 1], F32, tag="pcb")
        nc.tensor.matmul(pcb[:], R(sW1[:]), R(sb2[:]), start=True, stop=True)
        betas = [sb1]
        for i in range(2, 7):
            tmp = const.tile([H, 1], F32, tag=f"cbt{i}")
            nc.vector.tensor_scalar_mul(tmp[:], pcb[:], float(dt * _C[i - 1]))
            bt = const.tile([H, 1], F32, tag=f"beta{i}")
            nc.vector.tensor_add(bt[:], tmp[:], sb1[:])
            betas.append(bt)
        sbf = const.tile([H, 1], F32, tag="sbf")  # dt*b2 for the y update
        nc.vector.tensor_scalar_mul(sbf[:], sb2[:], float(dt))

        # h0 = x_last @ Wi + bi  (transposed: [H, BS])
        ph = psum.tile([H, BS], F32, tag="V")
        nc.tensor.matmul(ph[:], R(sWi[:]), R(sxT[:]), start=True, stop=True)
        y = state.tile([H, BS], F32, tag="y")
        nc.vector.tensor_scalar_add(y[:], ph[:], sbi[:])

        for _ in range(nsteps):
            V = psum.tile([H, BS], F32, tag="V")
            nc.tensor.matmul(V[:], R(sW1[:]), R(y[:]), start=True, stop=False)
            ts = []
            for i in range(1, 7):
                ti = tpool.tile([H, BS], F32, tag=f"t{i}")
                nc.scalar.activation(ti[:], V[:], Tanh, bias=betas[i - 1][:])
                ts.append(ti)
                if i < 6:
                    for j in range(1, i + 1):
                        last = i + 1 == 6 and j == i
                        nc.tensor.matmul(
                            V[:], R(G[(i + 1, j)][:]), R(ts[j - 1][:]),
                            start=False, stop=last,
                        )
            P2 = psum.tile([H, BS], F32, tag="P2")
            for n, j in enumerate((1, 3, 4, 5, 6)):
                nc.tensor.matmul(
                    P2[:], R(W2b[j][:]), R(ts[j - 1][:]), start=(n == 0), stop=(j == 6)
                )
            tmp = tpool.tile([H, BS], F32, tag="ktmp")
            nc.vector.tensor_scalar_add(tmp[:], P2[:], sbf[:])
            y2 = state.tile([H, BS], F32, tag="y")
            nc.vector.tensor_add(y2[:], y[:], tmp[:])
            y = y2

        # out = y @ Wo + bo  (transposed: [1, BS])
        po = psmall.tile([1, BS], F32, tag="po")
        nc.tensor.matmul(po[:], R(sWo[:]), R(y[:]), start=True, stop=True)
        so = const.tile([1, BS], F32, tag="so")
        nc.vector.tensor_scalar_add(so[:], po[:], sbo[:])
        nc.sync.dma_start(out[:], so[:])

    nc.finalize()
    return nc


_NC_CACHE: dict = {}


def _get_nc(nsteps: int = NSTEPS) -> bass.Bass:
    key = (nsteps, MM_DTYPE)
    if key not in _NC_CACHE:
        _NC_CACHE[key] = _build(nsteps)
    return _NC_CACHE[key]


def _in_maps(x, Wi, bi, W1, b1, W2, b2, Wo, bo):
    f = lambda a: np.ascontiguousarray(np.asarray(a), dtype=np.float32)
    x_lastT = f(np.asarray(x)[:, -1, :].T)  # [D, B]
    Wi, W1, W2, Wo = f(Wi), f(W1), f(W2), f(Wo)
    W2T = f(W2.T)
    bi_c, b1_c, b2_c = (f(v).reshape(H, 1) for v in (bi, b1, b2))
    Wo_c = Wo.reshape(H, 1)
    bo_c = f(bo).reshape(1, 1)
    maps = []
    for c in range(NCORES):
        maps.append({
            "xT": f(x_lastT[:, c * BS:(c + 1) * BS]),
            "Wi": Wi, "W1": W1, "W2": W2, "W2T": W2T,
            "bi": bi_c, "b1": b1_c, "b2": b2_c, "Wo": Wo_c, "bo": bo_c,
        })
    return maps


def kernel(x, Wi, bi, W1, b1, W2, b2, Wo, bo, _trace=False, _trace_kw=None):
    nc = _get_nc()
    maps = _in_maps(x, Wi, bi, W1, b1, W2, b2, Wo, bo)
    res = run_bass_kernel_spmd(
        nc, maps, list(range(NCORES)), trace=_trace, **(_trace_kw or {})
    )
    outv = np.concatenate([res.results[c]["out"][0] for c in range(NCORES)])
    out = outv.reshape(B, 1).astype(np.float32)
    if _trace:
        return out, res
    return out
